# revision 12
# baseline (speedup 1.0000x reference)
"""Trainium2 Bass kernel for top-2 MoE (B=8192, D=1024, E=8, F=1024).

Sharding: data-parallel over the batch across 8 NeuronCores. Each core gets
1024 tokens and the full (replicated) weights; no collectives are needed.

Per-core plan (sparse top-2 routing, capacity 384/expert):
  1. gating logits via fp32 matmuls (exact top-2 selection)
  2. top-2 + softmax with the DVE max8 instruction + ACT sigmoid
  3. bucket-by-expert on device: within-chunk exclusive cumsum via a
     strictly-triangular matmul, cross-chunk bases via a second tiny
     matmul, then indirect-DMA scatter of token ids into a DRAM perm list
  4. per expert: indirect-DMA gather of its token rows (cast to bf16),
     PE-transpose to d-major, then the two matmuls only over assigned
     tokens:  hT = gelu(w1.T @ xgT + b1),  y = hT.T @ w2  -> ybuf (DRAM)
  5. combine: per token-chunk, indirect-DMA gather of the two y rows,
     out = w0*ya + w1*yb + gw @ b2
"""

import os
import sys
import types

import numpy as np

_OPT_REPO = "/opt/trn_rl_repo"
if os.path.isdir(_OPT_REPO) and _OPT_REPO not in sys.path:
    sys.path.append(_OPT_REPO)

import concourse.bass as bass
import concourse.mybir as mybir
import concourse.tile as tile
from concourse import bacc
from concourse.bass_utils import run_bass_kernel_spmd
from concourse.masks import make_identity

P = 128
B = 8192
T = 1024  # tokens per core
D = 1024
E = 8
F = 1024
NDC = D // P
NFC = F // P
NTC = T // P
TH = 512
CAP = 384  # per-expert token capacity (mean 256, +9 sigma)
NCC = CAP // P  # capacity chunks per expert
NYR = E * CAP  # ybuf rows

f32 = mybir.dt.float32
bf16 = mybir.dt.bfloat16
i32 = mybir.dt.int32
u32 = mybir.dt.uint32
ALU = mybir.AluOpType
ACTF = mybir.ActivationFunctionType


def build_moe():
    nc = bacc.Bacc("TRN2", target_bir_lowering=False, debug=False)

    x = nc.dram_tensor("x", [T, D], f32, kind="ExternalInput")
    xt = nc.dram_tensor("xt", [D, T], f32, kind="ExternalInput")
    gatew = nc.dram_tensor("gatew", [D, E], f32, kind="ExternalInput")
    w1 = nc.dram_tensor("w1", [E, D, F], f32, kind="ExternalInput")
    b1 = nc.dram_tensor("b1", [E, F], f32, kind="ExternalInput")
    w2 = nc.dram_tensor("w2", [E, F, F], f32, kind="ExternalInput")
    b2 = nc.dram_tensor("b2", [E, F], f32, kind="ExternalInput")
    out = nc.dram_tensor("out", [T, F], f32, kind="ExternalOutput")
    gw = nc.dram_tensor("gw", [T, E], f32, kind="ExternalOutput")
    perm = nc.dram_tensor("perm", [NYR, 1], i32, kind="Internal")
    ybuf = nc.dram_tensor("ybuf", [NYR, F], bf16, kind="Internal")

    with tile.TileContext(nc) as tc:
        with (
            tc.tile_pool(name="const", bufs=1) as const_pool,
            tc.tile_pool(name="gate", bufs=2) as gate_pool,
            tc.tile_pool(name="route", bufs=1) as route_pool,
            tc.tile_pool(name="w1", bufs=2) as w1_pool,
            tc.tile_pool(name="w2", bufs=2) as w2_pool,
            tc.tile_pool(name="b1", bufs=2) as b1_pool,
            tc.tile_pool(name="xg", bufs=6) as xg_pool,
            tc.tile_pool(name="xgt", bufs=2) as xgt_pool,
            tc.tile_pool(name="ht", bufs=2) as h_pool,
            tc.tile_pool(name="yst", bufs=3) as yst_pool,
            tc.tile_pool(name="comb", bufs=2) as comb_pool,
        ):
            # ---- constants ----
            gatew_sb = const_pool.tile([P, NDC * E], f32)
            for dc in range(NDC):
                nc.sync.dma_start(
                    gatew_sb[:, dc * E : (dc + 1) * E],
                    gatew[dc * P : (dc + 1) * P, :],
                )
            b2_sb = const_pool.tile([E, F], bf16)
            nc.gpsimd.dma_start(b2_sb[:], b2[:, :])
            ident_bf = const_pool.tile([P, P], bf16)
            make_identity(nc, ident_bf[:])
            ident_f = const_pool.tile([P, P], f32)
            make_identity(nc, ident_f[:])
            # L[p, f] = 1 if f > p (strictly upper triangular ones), bf16
            iota_row = const_pool.tile([P, P], f32)
            nc.gpsimd.iota(
                iota_row[:], [[1, P]], channel_multiplier=0,
                allow_small_or_imprecise_dtypes=True,
            )
            iota_col = const_pool.tile([P, 1], f32)
            nc.gpsimd.iota(
                iota_col[:], [[1, 1]], channel_multiplier=1,
                allow_small_or_imprecise_dtypes=True,
            )
            ltri = const_pool.tile([P, P], bf16)
            nc.vector.tensor_scalar(
                ltri[:], iota_row[:], iota_col[:, 0:1], None, op0=ALU.is_gt
            )
            ltri_f = const_pool.tile([P, P], f32)
            nc.vector.tensor_scalar(
                ltri_f[:], iota_row[:], iota_col[:, 0:1], None, op0=ALU.is_gt
            )
            ones_col = const_pool.tile([P, 1], bf16)
            nc.vector.memset(ones_col[:], 1.0)
            ones_row = const_pool.tile([1, P], f32)
            nc.vector.memset(ones_row[:], 1.0)
            zeros24 = const_pool.tile([P, 24], i32)
            nc.vector.memset(zeros24[:], 0)

            # init perm with zeros (padding slots gather row 0 harmlessly)
            for a in range(NYR // P):
                nc.sync.dma_start(perm[a * P : (a + 1) * P, :], zeros24[:, a : a + 1])

            # per-token-chunk routing state kept in SBUF
            gw_sb = [
                gate_pool.tile([P, E], f32, tag=f"gw{t}", name=f"gw_sb{t}")
                for t in range(NTC)
            ]
            logit_sb = [
                gate_pool.tile([P, E], f32, tag=f"lg{t}", name=f"logit{t}")
                for t in range(NTC)
            ]
            amask = [
                gate_pool.tile([P, E], bf16, tag=f"am{t}", name=f"amask{t}")
                for t in range(NTC)
            ]
            eqr = [
                [
                    gate_pool.tile([P, E], f32, tag=f"eq{t}_{k}", name=f"eqr{t}_{k}")
                    for k in range(2)
                ]
                for t in range(NTC)
            ]
            wtop = [
                [
                    gate_pool.tile([P, 1], f32, tag=f"wt{t}_{k}", name=f"wtop{t}_{k}")
                    for k in range(2)
                ]
                for t in range(NTC)
            ]
            idxf = [
                [
                    gate_pool.tile([P, 1], f32, tag=f"ix{t}_{k}", name=f"idxf{t}_{k}")
                    for k in range(2)
                ]
                for t in range(NTC)
            ]
            pos_i = [
                [
                    gate_pool.tile([P, 1], i32, tag=f"ps{t}_{k}", name=f"pos{t}_{k}")
                    for k in range(2)
                ]
                for t in range(NTC)
            ]

            # weight load helper: prefetch so the GPSIMD queue never blocks
            # an expert's gathers behind its own weight loads
            wload = {}

            def emit_wloads(e):
                b1_sb = b1_pool.tile([P, NFC], f32, name=f"b1_sb{e}")
                for fc in range(NFC):
                    nc.sync.dma_start(
                        b1_sb[:, fc : fc + 1], b1[e, fc * P : (fc + 1) * P, None]
                    )
                w1_sb = [
                    w1_pool.tile([P, F], bf16, tag=f"w1_{dc}", name=f"w1_sb{e}_{dc}")
                    for dc in range(NDC)
                ]
                for dc in range(NDC):
                    nc.gpsimd.dma_start(w1_sb[dc][:], w1[e, dc * P : (dc + 1) * P, :])
                w2_sb = [
                    w2_pool.tile([P, F], bf16, tag=f"w2_{fc}", name=f"w2_sb{e}_{fc}")
                    for fc in range(NFC)
                ]
                for fc in range(NFC):
                    nc.gpsimd.dma_start(w2_sb[fc][:], w2[e, fc * P : (fc + 1) * P, :])
                wload[e] = (b1_sb, w1_sb, w2_sb)

            emit_wloads(0)

            # ---- gating logits (fp32, streamed xt chunks) ----
            with tc.tile_pool(name="plog", bufs=1, space="PSUM") as plog_pool:
                plogs = [
                    plog_pool.tile([P, E], f32, tag=f"plog{t}", name=f"plog{t}")
                    for t in range(NTC)
                ]
                with tc.tile_pool(name="xtg", bufs=2) as xtg_pool:
                    for dc in range(NDC):
                        xtg = xtg_pool.tile([P, T], f32)
                        nc.sync.dma_start(xtg[:], xt[dc * P : (dc + 1) * P, :])
                        for t in range(NTC):
                            nc.tensor.matmul(
                                plogs[t][:],
                                lhsT=xtg[:, t * P : (t + 1) * P],
                                rhs=gatew_sb[:, dc * E : (dc + 1) * E],
                                start=(dc == 0),
                                stop=(dc == NDC - 1),
                            )
                for t in range(NTC):
                    nc.vector.tensor_copy(logit_sb[t][:], plogs[t][:])

            # ---- top-2 selection + gate weights ----
            for t in range(NTC):
                logits = logit_sb[t]
                m8 = gate_pool.tile([P, 8], f32)
                nc.vector.max(m8[:], logits[:])
                idx8 = gate_pool.tile([P, 8], u32)
                nc.vector.max_index(idx8[:], m8[:], logits[:])
                nc.vector.tensor_copy(idxf[t][0][:], idx8[:, 0:1])
                nc.vector.tensor_copy(idxf[t][1][:], idx8[:, 1:2])
                diff = gate_pool.tile([P, 1], f32)
                nc.vector.tensor_sub(diff[:], m8[:, 1:2], m8[:, 0:1])
                nc.scalar.activation(wtop[t][1][:], diff[:], ACTF.Sigmoid)
                nc.vector.tensor_scalar(
                    wtop[t][0][:], wtop[t][1][:], -1.0, 1.0, op0=ALU.mult, op1=ALU.add
                )
                nc.vector.tensor_scalar(
                    eqr[t][0][:], logits[:], m8[:, 0:1], None, op0=ALU.is_equal
                )
                nc.vector.tensor_scalar(
                    eqr[t][1][:], logits[:], m8[:, 1:2], None, op0=ALU.is_equal
                )
                nc.vector.tensor_add(amask[t][:], eqr[t][0][:], eqr[t][1][:])
                e1w = gate_pool.tile([P, E], f32)
                nc.vector.tensor_scalar_mul(e1w[:], eqr[t][0][:], wtop[t][0][:, 0:1])
                e2w = gate_pool.tile([P, E], f32)
                nc.vector.tensor_scalar_mul(e2w[:], eqr[t][1][:], wtop[t][1][:, 0:1])
                nc.vector.tensor_add(gw_sb[t][:], e1w[:], e2w[:])
                nc.sync.dma_start(gw[t * P : (t + 1) * P, :], gw_sb[t][:])

            # ---- slot assignment (exclusive cumsum over tokens per expert) ----
            counts_sb = route_pool.tile([E, E], f32)  # [chunk, expert]
            with tc.tile_pool(name="pcnt", bufs=4, space="PSUM") as pcnt_pool:
                for t in range(NTC):
                    pcnt = pcnt_pool.tile([1, E], f32)
                    nc.tensor.matmul(
                        pcnt[:], lhsT=ones_col[:], rhs=amask[t][:],
                        start=True, stop=True,
                    )
                    cstage = route_pool.tile([1, E], f32, tag=f"cs{t}", name=f"cstage{t}")
                    nc.vector.tensor_copy(cstage[:], pcnt[:])
                    nc.sync.dma_start(counts_sb[t : t + 1, :], cstage[:])
            base_row = [
                route_pool.tile([1, E], f32, tag=f"br{t}", name=f"base_row{t}")
                for t in range(NTC)
            ]
            with tc.tile_pool(name="pbase", bufs=1, space="PSUM") as pbase_pool:
                pbase = pbase_pool.tile([E, E], f32)
                nc.tensor.matmul(
                    pbase[:], lhsT=ltri_f[:E, :E], rhs=counts_sb[:],
                    start=True, stop=True,
                )
                base8 = route_pool.tile([E, E], f32)
                nc.vector.tensor_copy(base8[:], pbase[:])
                for t in range(NTC):
                    nc.sync.dma_start(base_row[t][:], base8[t : t + 1, :])

            with tc.tile_pool(name="pslot", bufs=4, space="PSUM") as pslot_pool:
                for t in range(NTC):
                    pslot = pslot_pool.tile([P, E], f32)
                    nc.tensor.matmul(
                        pslot[:], lhsT=ltri[:], rhs=amask[t][:],
                        start=True, stop=False,
                    )
                    nc.tensor.matmul(
                        pslot[:], lhsT=ones_row[:], rhs=base_row[t][:],
                        start=False, stop=True,
                    )
                    slot = gate_pool.tile([P, E], f32)
                    nc.vector.tensor_copy(slot[:], pslot[:])
                    tokid = gate_pool.tile([P, 1], i32)
                    nc.vector.tensor_scalar(
                        tokid[:], iota_col[:], 1.0, float(t * P),
                        op0=ALU.mult, op1=ALU.add,
                    )
                    for k in range(2):
                        sel = gate_pool.tile([P, E], f32)
                        nc.vector.tensor_mul(sel[:], slot[:], eqr[t][k][:])
                        ssel = gate_pool.tile([P, 1], f32)
                        nc.vector.tensor_reduce(
                            ssel[:], sel[:], axis=mybir.AxisListType.X, op=ALU.add
                        )
                        posf = gate_pool.tile([P, 1], f32)
                        nc.vector.scalar_tensor_tensor(
                            out=posf[:], in0=idxf[t][k][:], scalar=float(CAP),
                            in1=ssel[:], op0=ALU.mult, op1=ALU.add,
                        )
                        nc.vector.tensor_copy(pos_i[t][k][:], posf[:])
                        nc.gpsimd.indirect_dma_start(
                            out=perm[:, :],
                            out_offset=bass.IndirectOffsetOnAxis(
                                ap=pos_i[t][k][:, :1], axis=0
                            ),
                            in_=tokid[:, :1],
                            in_offset=None,
                            bounds_check=NYR - 1,
                            oob_is_err=False,
                        )

            # ---- per-expert sparse MLP ----
            with (
                tc.tile_pool(name="ph", bufs=3, space="PSUM") as ph_pool,
                tc.tile_pool(name="py", bufs=3, space="PSUM") as py_pool,
            ):
                for e in range(E):
                    b1_sb, w1_sb, w2_sb = wload[e]

                    # gather this expert's token rows (cast fp32 -> bf16)
                    xg = []
                    for j in range(NCC):
                        pslice = route_pool.tile(
                            [P, 1], i32, tag="pslice", name=f"pslice{e}_{j}"
                        )
                        nc.sync.dma_start(
                            pslice[:],
                            perm[(e * CAP + j * P) : (e * CAP + (j + 1) * P), :],
                        )
                        xgj = xg_pool.tile([P, D], bf16, tag="xg", name=f"xg{e}_{j}")
                        nc.gpsimd.indirect_dma_start(
                            out=xgj[:],
                            out_offset=None,
                            in_=x[:, :],
                            in_offset=bass.IndirectOffsetOnAxis(
                                ap=pslice[:, :1], axis=0
                            ),
                        )
                        xg.append(xgj)

                    # transpose gathered rows to d-major via HWDGE DMA xbar:
                    # xgt layout [p=d%128, dc, j, s]
                    xgt = xgt_pool.tile([P, NDC * NCC * P], bf16, tag="xgt",
                                        name=f"xgt{e}")
                    xgt_v = xgt[:].rearrange(
                        "p (d j s) -> p d j s", d=NDC, j=NCC
                    )
                    for j in range(NCC):
                        nc.scalar.dma_start_transpose(xgt_v[:, :, j, :], xg[j][:])

                    # prefetch next expert's weights now (behind the gathers
                    # on the gpsimd queue, ahead of compute)
                    if e + 1 < E:
                        emit_wloads(e + 1)

                    # phase A: hT = gelu(w1.T @ xgT + b1)
                    ht = [
                        h_pool.tile([P, CAP], bf16, tag=f"ht{fc}", name=f"ht{e}_{fc}")
                        for fc in range(NFC)
                    ]
                    for fc in range(NFC):
                        fsl = slice(fc * P, (fc + 1) * P)
                        ph = ph_pool.tile([P, CAP], f32)
                        for dc in range(NDC):
                            nc.tensor.matmul(
                                ph[:],
                                lhsT=w1_sb[dc][:, fsl],
                                rhs=xgt_v[:, dc, :, :],
                                start=(dc == 0),
                                stop=(dc == NDC - 1),
                            )
                        nc.scalar.activation(
                            ht[fc][:], ph[:], ACTF.Gelu, bias=b1_sb[:, fc : fc + 1]
                        )

                    # phase B: y = hT.T @ w2 -> ybuf rows (bf16)
                    for j in range(NCC):
                        jsl = slice(j * P, (j + 1) * P)
                        yst = yst_pool.tile([P, F], bf16, tag="yst", name=f"yst{e}_{j}")
                        for gh in range(2):
                            gsl = slice(gh * TH, (gh + 1) * TH)
                            py = py_pool.tile([P, TH], f32)
                            for fc in range(NFC):
                                nc.tensor.matmul(
                                    py[:],
                                    lhsT=ht[fc][:, jsl],
                                    rhs=w2_sb[fc][:, gsl],
                                    start=(fc == 0),
                                    stop=(fc == NFC - 1),
                                )
                            if gh == 0:
                                nc.scalar.copy(yst[:, gsl], py[:])
                            else:
                                nc.vector.tensor_copy(yst[:, gsl], py[:])
                        nc.sync.dma_start(
                            ybuf[(e * CAP + j * P) : (e * CAP + (j + 1) * P), :],
                            yst[:],
                        )

            # ---- combine: out = w0*ya + w1*yb + gw @ b2 ----
            with tc.tile_pool(name="pb2", bufs=2, space="PSUM") as pb2_pool:
                for t in range(NTC):
                    tsl = slice(t * P, (t + 1) * P)
                    ya = comb_pool.tile([P, F], bf16, tag="ya", name=f"ya{t}")
                    nc.gpsimd.indirect_dma_start(
                        out=ya[:],
                        out_offset=None,
                        in_=ybuf[:, :],
                        in_offset=bass.IndirectOffsetOnAxis(
                            ap=pos_i[t][0][:, :1], axis=0
                        ),
                    )
                    yb = comb_pool.tile([P, F], bf16, tag="yb", name=f"yb{t}")
                    nc.gpsimd.indirect_dma_start(
                        out=yb[:],
                        out_offset=None,
                        in_=ybuf[:, :],
                        in_offset=bass.IndirectOffsetOnAxis(
                            ap=pos_i[t][1][:, :1], axis=0
                        ),
                    )
                    gwt = gate_pool.tile([E, P], bf16)
                    ptr2 = pb2_pool.tile([E, P], f32, tag="ptr2", name=f"ptr2_{t}")
                    nc.tensor.transpose(ptr2[:], gw_sb[t][:], ident_f[:])
                    nc.vector.tensor_copy(gwt[:], ptr2[:])
                    osb = comb_pool.tile([P, F], f32, tag="osb", name=f"osb{t}")
                    tmp = comb_pool.tile([P, F], f32, tag="tmp", name=f"tmp{t}")
                    for gh in range(2):
                        gsl = slice(gh * TH, (gh + 1) * TH)
                        pb = pb2_pool.tile([P, TH], f32, tag="pb", name=f"pb{t}_{gh}")
                        nc.tensor.matmul(
                            pb[:], lhsT=gwt[:], rhs=b2_sb[:, gsl], start=True, stop=True
                        )
                        nc.vector.scalar_tensor_tensor(
                            out=tmp[:, gsl], in0=ya[:, gsl],
                            scalar=wtop[t][0][:, 0:1], in1=pb[:],
                            op0=ALU.mult, op1=ALU.add,
                        )
                        nc.vector.scalar_tensor_tensor(
                            out=osb[:, gsl], in0=yb[:, gsl],
                            scalar=wtop[t][1][:, 0:1], in1=tmp[:, gsl],
                            op0=ALU.mult, op1=ALU.add,
                        )
                    nc.sync.dma_start(out[tsl, :], osb[:])

    nc.compile()
    return nc


_NC = None


def _get_nc():
    global _NC
    if _NC is None:
        _NC = build_moe()
    return _NC


def _maybe_enable_trace():
    """Register the NTFF profile hook (missing antenv.axon_hooks shim)."""
    try:
        import antenv

        if "antenv.axon_hooks" not in sys.modules:
            hooks = types.ModuleType("antenv.axon_hooks")
            hooks._hook = None
            hooks.set_axon_ntff_profile_hook = lambda h: setattr(hooks, "_hook", h)
            hooks.get_axon_ntff_profile_hook = lambda: hooks._hook
            sys.modules["antenv.axon_hooks"] = hooks
            antenv.axon_hooks = hooks
            from trn_agent_boot.trn_boot import _ntff_profile_via_ctypes

            hooks.set_axon_ntff_profile_hook(
                _ntff_profile_via_ctypes("/opt/axon/libaxon_pjrt.so")
            )
        return True
    except Exception:
        return False


LAST_EXEC_TIME_NS = None
LAST_RESULT = None


def kernel(x, gate_w, w1, b1, w2, b2):
    global LAST_EXEC_TIME_NS, LAST_RESULT
    x = np.ascontiguousarray(np.asarray(x, dtype=np.float32))
    gate_w = np.ascontiguousarray(np.asarray(gate_w, dtype=np.float32))
    w1 = np.ascontiguousarray(np.asarray(w1, dtype=np.float32))
    b1 = np.ascontiguousarray(np.asarray(b1, dtype=np.float32))
    w2 = np.ascontiguousarray(np.asarray(w2, dtype=np.float32))
    b2 = np.ascontiguousarray(np.asarray(b2, dtype=np.float32))

    trace = bool(os.environ.get("BASS_MOE_TRACE"))
    if trace:
        trace = _maybe_enable_trace()

    nc = _get_nc()
    in_maps = []
    for c in range(8):
        xs = x[c * T : (c + 1) * T]
        in_maps.append(
            {
                "x": xs,
                "xt": np.ascontiguousarray(xs.T),
                "gatew": gate_w,
                "w1": w1,
                "b1": b1,
                "w2": w2,
                "b2": b2,
            }
        )
    res = run_bass_kernel_spmd(nc, in_maps, list(range(8)), trace=trace)
    LAST_RESULT = res
    LAST_EXEC_TIME_NS = res.exec_time_ns
    out = np.concatenate([res.results[c]["out"] for c in range(8)], axis=0)
    gws = np.concatenate([res.results[c]["gw"] for c in range(8)], axis=0)
    return out, gws


# revision 14
# speedup vs baseline: 1.4108x; 1.4108x over previous
"""Trainium2 Bass kernel for top-2 MoE (B=8192, D=1024, E=8, F=1024).

Sharding: data-parallel over the batch across 8 NeuronCores. Each core gets
1024 tokens and the full (replicated) weights; no collectives are needed.

Per-core plan (sparse top-2 routing, capacity 384/expert):
  1. gating logits via fp32 matmuls (exact top-2 selection)
  2. top-2 + softmax with the DVE max8 instruction + ACT sigmoid
  3. bucket-by-expert on device: within-chunk exclusive cumsum via a
     strictly-triangular matmul, cross-chunk bases via a second tiny
     matmul, then indirect-DMA scatter of token ids into a DRAM perm list
  4. per expert: indirect-DMA gather of its token rows (cast to bf16),
     PE-transpose to d-major, then the two matmuls only over assigned
     tokens:  hT = gelu(w1.T @ xgT + b1),  y = hT.T @ w2  -> ybuf (DRAM)
  5. combine: per token-chunk, indirect-DMA gather of the two y rows,
     out = w0*ya + w1*yb + gw @ b2
"""

import os
import sys
import types

import numpy as np

_OPT_REPO = "/opt/trn_rl_repo"
if os.path.isdir(_OPT_REPO) and _OPT_REPO not in sys.path:
    sys.path.append(_OPT_REPO)

import concourse.bass as bass
import concourse.mybir as mybir
import concourse.tile as tile
from concourse import bacc
from concourse.bass_utils import run_bass_kernel_spmd
from concourse.masks import make_identity

P = 128
B = 8192
T = 1024  # tokens per core
D = 1024
E = 8
F = 1024
NDC = D // P
NFC = F // P
NTC = T // P
TH = 512
CAP = 384  # per-expert token capacity (mean 256, +9 sigma)
NCC = CAP // P  # capacity chunks per expert
NYR = E * CAP  # ybuf rows

f32 = mybir.dt.float32
bf16 = mybir.dt.bfloat16
i32 = mybir.dt.int32
u32 = mybir.dt.uint32
ALU = mybir.AluOpType
ACTF = mybir.ActivationFunctionType


def build_moe():
    nc = bacc.Bacc("TRN2", target_bir_lowering=False, debug=False)

    x = nc.dram_tensor("x", [T, D], f32, kind="ExternalInput")
    xt = nc.dram_tensor("xt", [D, T], f32, kind="ExternalInput")
    gatew = nc.dram_tensor("gatew", [D, E], f32, kind="ExternalInput")
    w1 = nc.dram_tensor("w1", [E, D, F], f32, kind="ExternalInput")
    b1 = nc.dram_tensor("b1", [E, F], f32, kind="ExternalInput")
    w2 = nc.dram_tensor("w2", [E, F, F], f32, kind="ExternalInput")
    b2 = nc.dram_tensor("b2", [E, F], f32, kind="ExternalInput")
    out = nc.dram_tensor("out", [T, F], f32, kind="ExternalOutput")
    gw = nc.dram_tensor("gw", [T, E], f32, kind="ExternalOutput")
    # perm[slot] = (token id as float, gate weight); sentinel token id T
    # makes padded slots fall out of bounds_check on both gather and
    # scatter-add, so they are silently dropped.
    perm = nc.dram_tensor("perm", [NYR, 2], f32, kind="Internal")

    with tile.TileContext(nc) as tc:
        with (
            tc.tile_pool(name="const", bufs=1) as const_pool,
            tc.tile_pool(name="gate", bufs=2) as gate_pool,
            tc.tile_pool(name="route", bufs=1) as route_pool,
            tc.tile_pool(name="w1", bufs=2) as w1_pool,
            tc.tile_pool(name="w2", bufs=2) as w2_pool,
            tc.tile_pool(name="b1", bufs=2) as b1_pool,
            tc.tile_pool(name="xg", bufs=6) as xg_pool,
            tc.tile_pool(name="xgt", bufs=2) as xgt_pool,
            tc.tile_pool(name="ht", bufs=2) as h_pool,
            tc.tile_pool(name="yst", bufs=3) as yst_pool,
            tc.tile_pool(name="comb", bufs=2) as comb_pool,
        ):
            # ---- constants ----
            gatew_sb = const_pool.tile([P, NDC * E], f32)
            for dc in range(NDC):
                nc.sync.dma_start(
                    gatew_sb[:, dc * E : (dc + 1) * E],
                    gatew[dc * P : (dc + 1) * P, :],
                )
            b2_sb = const_pool.tile([E, F], bf16)
            nc.gpsimd.dma_start(b2_sb[:], b2[:, :])
            ident_bf = const_pool.tile([P, P], bf16)
            make_identity(nc, ident_bf[:])
            ident_f = const_pool.tile([P, P], f32)
            make_identity(nc, ident_f[:])
            iota_row = const_pool.tile([P, P], f32)
            nc.gpsimd.iota(
                iota_row[:], [[1, P]], channel_multiplier=0,
                allow_small_or_imprecise_dtypes=True,
            )
            iota_col = const_pool.tile([P, 1], f32)
            nc.gpsimd.iota(
                iota_col[:], [[1, 1]], channel_multiplier=1,
                allow_small_or_imprecise_dtypes=True,
            )
            ltri = const_pool.tile([P, P], bf16)
            nc.vector.tensor_scalar(
                ltri[:], iota_row[:], iota_col[:, 0:1], None, op0=ALU.is_gt
            )
            ltri_f = const_pool.tile([P, P], f32)
            nc.vector.tensor_scalar(
                ltri_f[:], iota_row[:], iota_col[:, 0:1], None, op0=ALU.is_gt
            )
            ones_col = const_pool.tile([P, 1], bf16)
            nc.vector.memset(ones_col[:], 1.0)
            ones_row = const_pool.tile([1, P], f32)
            nc.vector.memset(ones_row[:], 1.0)
            sent48 = const_pool.tile([P, NYR // P * 2], f32)
            nc.vector.memset(sent48[:], float(T))

            # init perm with the sentinel
            for a in range(NYR // P):
                nc.sync.dma_start(
                    perm[a * P : (a + 1) * P, :], sent48[:, 2 * a : 2 * a + 2]
                )

            # per-token-chunk routing state kept in SBUF
            gw_sb = [
                gate_pool.tile([P, E], f32, tag=f"gw{t}", name=f"gw_sb{t}")
                for t in range(NTC)
            ]
            logit_sb = [
                gate_pool.tile([P, E], f32, tag=f"lg{t}", name=f"logit{t}")
                for t in range(NTC)
            ]
            amask = [
                gate_pool.tile([P, E], bf16, tag=f"am{t}", name=f"amask{t}")
                for t in range(NTC)
            ]
            eqr = [
                [
                    gate_pool.tile([P, E], f32, tag=f"eq{t}_{k}", name=f"eqr{t}_{k}")
                    for k in range(2)
                ]
                for t in range(NTC)
            ]
            wtop = [
                [
                    gate_pool.tile([P, 1], f32, tag=f"wt{t}_{k}", name=f"wtop{t}_{k}")
                    for k in range(2)
                ]
                for t in range(NTC)
            ]
            idxf = [
                [
                    gate_pool.tile([P, 1], f32, tag=f"ix{t}_{k}", name=f"idxf{t}_{k}")
                    for k in range(2)
                ]
                for t in range(NTC)
            ]

            # weight load helper: prefetch so the GPSIMD queue never blocks
            # an expert's gathers behind its own weight loads
            wload = {}

            def emit_wloads(e):
                b1_sb = b1_pool.tile([P, NFC], f32, name=f"b1_sb{e}")
                for fc in range(NFC):
                    nc.sync.dma_start(
                        b1_sb[:, fc : fc + 1], b1[e, fc * P : (fc + 1) * P, None]
                    )
                w1_sb = [
                    w1_pool.tile([P, F], bf16, tag=f"w1_{dc}", name=f"w1_sb{e}_{dc}")
                    for dc in range(NDC)
                ]
                for dc in range(NDC):
                    nc.gpsimd.dma_start(w1_sb[dc][:], w1[e, dc * P : (dc + 1) * P, :])
                w2_sb = [
                    w2_pool.tile([P, F], bf16, tag=f"w2_{fc}", name=f"w2_sb{e}_{fc}")
                    for fc in range(NFC)
                ]
                for fc in range(NFC):
                    nc.gpsimd.dma_start(w2_sb[fc][:], w2[e, fc * P : (fc + 1) * P, :])
                wload[e] = (b1_sb, w1_sb, w2_sb)

            emit_wloads(0)

            # ---- gating logits (fp32, streamed xt chunks) ----
            with tc.tile_pool(name="plog", bufs=1, space="PSUM") as plog_pool:
                plogs = [
                    plog_pool.tile([P, E], f32, tag=f"plog{t}", name=f"plog{t}")
                    for t in range(NTC)
                ]
                with tc.tile_pool(name="xtg", bufs=2) as xtg_pool:
                    for dc in range(NDC):
                        xtg = xtg_pool.tile([P, T], f32)
                        nc.sync.dma_start(xtg[:], xt[dc * P : (dc + 1) * P, :])
                        for t in range(NTC):
                            nc.tensor.matmul(
                                plogs[t][:],
                                lhsT=xtg[:, t * P : (t + 1) * P],
                                rhs=gatew_sb[:, dc * E : (dc + 1) * E],
                                start=(dc == 0),
                                stop=(dc == NDC - 1),
                            )
                for t in range(NTC):
                    nc.vector.tensor_copy(logit_sb[t][:], plogs[t][:])

            # ---- top-2 selection + gate weights ----
            for t in range(NTC):
                logits = logit_sb[t]
                m8 = gate_pool.tile([P, 8], f32)
                nc.vector.max(m8[:], logits[:])
                idx8 = gate_pool.tile([P, 8], u32)
                nc.vector.max_index(idx8[:], m8[:], logits[:])
                nc.vector.tensor_copy(idxf[t][0][:], idx8[:, 0:1])
                nc.vector.tensor_copy(idxf[t][1][:], idx8[:, 1:2])
                diff = gate_pool.tile([P, 1], f32)
                nc.vector.tensor_sub(diff[:], m8[:, 1:2], m8[:, 0:1])
                nc.scalar.activation(wtop[t][1][:], diff[:], ACTF.Sigmoid)
                nc.vector.tensor_scalar(
                    wtop[t][0][:], wtop[t][1][:], -1.0, 1.0, op0=ALU.mult, op1=ALU.add
                )
                nc.vector.tensor_scalar(
                    eqr[t][0][:], logits[:], m8[:, 0:1], None, op0=ALU.is_equal
                )
                nc.vector.tensor_scalar(
                    eqr[t][1][:], logits[:], m8[:, 1:2], None, op0=ALU.is_equal
                )
                nc.vector.tensor_add(amask[t][:], eqr[t][0][:], eqr[t][1][:])
                e1w = gate_pool.tile([P, E], f32)
                nc.vector.tensor_scalar_mul(e1w[:], eqr[t][0][:], wtop[t][0][:, 0:1])
                e2w = gate_pool.tile([P, E], f32)
                nc.vector.tensor_scalar_mul(e2w[:], eqr[t][1][:], wtop[t][1][:, 0:1])
                nc.vector.tensor_add(gw_sb[t][:], e1w[:], e2w[:])
                nc.sync.dma_start(gw[t * P : (t + 1) * P, :], gw_sb[t][:])

            # ---- slot assignment (exclusive cumsum over tokens per expert) ----
            counts_sb = route_pool.tile([E, E], f32)  # [chunk, expert]
            with tc.tile_pool(name="pcnt", bufs=4, space="PSUM") as pcnt_pool:
                for t in range(NTC):
                    pcnt = pcnt_pool.tile([1, E], f32)
                    nc.tensor.matmul(
                        pcnt[:], lhsT=ones_col[:], rhs=amask[t][:],
                        start=True, stop=True,
                    )
                    cstage = route_pool.tile([1, E], f32, tag=f"cs{t}", name=f"cstage{t}")
                    nc.vector.tensor_copy(cstage[:], pcnt[:])
                    nc.sync.dma_start(counts_sb[t : t + 1, :], cstage[:])
            base_row = [
                route_pool.tile([1, E], f32, tag=f"br{t}", name=f"base_row{t}")
                for t in range(NTC)
            ]
            with tc.tile_pool(name="pbase", bufs=1, space="PSUM") as pbase_pool:
                pbase = pbase_pool.tile([E, E], f32)
                nc.tensor.matmul(
                    pbase[:], lhsT=ltri_f[:E, :E], rhs=counts_sb[:],
                    start=True, stop=True,
                )
                base8 = route_pool.tile([E, E], f32)
                nc.vector.tensor_copy(base8[:], pbase[:])
                for t in range(NTC):
                    nc.sync.dma_start(base_row[t][:], base8[t : t + 1, :])

            with tc.tile_pool(name="pslot", bufs=4, space="PSUM") as pslot_pool:
                for t in range(NTC):
                    pslot = pslot_pool.tile([P, E], f32)
                    nc.tensor.matmul(
                        pslot[:], lhsT=ltri[:], rhs=amask[t][:],
                        start=True, stop=False,
                    )
                    nc.tensor.matmul(
                        pslot[:], lhsT=ones_row[:], rhs=base_row[t][:],
                        start=False, stop=True,
                    )
                    slot = gate_pool.tile([P, E], f32)
                    nc.vector.tensor_copy(slot[:], pslot[:])
                    tokf = gate_pool.tile([P, 1], f32)
                    nc.vector.tensor_scalar(
                        tokf[:], iota_col[:], 1.0, float(t * P),
                        op0=ALU.mult, op1=ALU.add,
                    )
                    for k in range(2):
                        sel = gate_pool.tile([P, E], f32)
                        nc.vector.tensor_mul(sel[:], slot[:], eqr[t][k][:])
                        ssel = gate_pool.tile([P, 1], f32)
                        nc.vector.tensor_reduce(
                            ssel[:], sel[:], axis=mybir.AxisListType.X, op=ALU.add
                        )
                        posf = gate_pool.tile([P, 1], f32)
                        nc.vector.scalar_tensor_tensor(
                            out=posf[:], in0=idxf[t][k][:], scalar=float(CAP),
                            in1=ssel[:], op0=ALU.mult, op1=ALU.add,
                        )
                        # capacity overflow -> push out of bounds so the
                        # scatter drops it instead of corrupting a neighbour
                        ovf = gate_pool.tile([P, 1], f32)
                        nc.vector.tensor_scalar(
                            ovf[:], ssel[:], float(CAP), None, op0=ALU.is_ge
                        )
                        nc.vector.scalar_tensor_tensor(
                            out=posf[:], in0=ovf[:], scalar=1.0e6,
                            in1=posf[:], op0=ALU.mult, op1=ALU.add,
                        )
                        pos_int = gate_pool.tile([P, 1], i32)
                        nc.vector.tensor_copy(pos_int[:], posf[:])
                        pair = gate_pool.tile([P, 2], f32)
                        nc.vector.tensor_copy(pair[:, 0:1], tokf[:])
                        nc.vector.tensor_copy(pair[:, 1:2], wtop[t][k][:])
                        nc.gpsimd.indirect_dma_start(
                            out=perm[:, :],
                            out_offset=bass.IndirectOffsetOnAxis(
                                ap=pos_int[:, :1], axis=0
                            ),
                            in_=pair[:, :],
                            in_offset=None,
                            bounds_check=NYR - 1,
                            oob_is_err=False,
                        )

            # ---- init out with the gw @ b2 term ----
            with tc.tile_pool(name="pb2", bufs=2, space="PSUM") as pb2_pool:
                for t in range(NTC):
                    gwt = gate_pool.tile([E, P], f32)
                    ptr2 = pb2_pool.tile([E, P], f32, tag="ptr2", name=f"ptr2_{t}")
                    nc.tensor.transpose(ptr2[:], gw_sb[t][:], ident_f[:])
                    nc.vector.tensor_copy(gwt[:], ptr2[:])
                    gwt_bf = gate_pool.tile([E, P], bf16)
                    nc.vector.tensor_copy(gwt_bf[:], gwt[:])
                    osb = comb_pool.tile([P, F], f32, tag="osb", name=f"osb{t}")
                    for gh in range(2):
                        gsl = slice(gh * TH, (gh + 1) * TH)
                        pb = pb2_pool.tile([P, TH], f32, tag="pb", name=f"pb{t}_{gh}")
                        nc.tensor.matmul(
                            pb[:], lhsT=gwt_bf[:], rhs=b2_sb[:, gsl],
                            start=True, stop=True,
                        )
                        nc.vector.tensor_copy(osb[:, gsl], pb[:])
                    nc.sync.dma_start(out[t * P : (t + 1) * P, :], osb[:])

            # ---- per-expert sparse MLP, scatter-add into out ----
            with (
                tc.tile_pool(name="ptr", bufs=2, space="PSUM") as ptr_pool,
                tc.tile_pool(name="ph", bufs=3, space="PSUM") as ph_pool,
                tc.tile_pool(name="py", bufs=3, space="PSUM") as py_pool,
            ):
                for e in range(E):
                    b1_sb, w1_sb, w2_sb = wload[e]

                    # gather this expert's token rows (cast fp32 -> bf16)
                    xg = []
                    idx_i = []
                    pgw = []
                    for j in range(NCC):
                        pslice = route_pool.tile(
                            [P, 2], f32, tag="pslice", name=f"pslice{e}_{j}", bufs=8
                        )
                        nc.sync.dma_start(
                            pslice[:],
                            perm[(e * CAP + j * P) : (e * CAP + (j + 1) * P), :],
                        )
                        idxj = route_pool.tile([P, 1], i32, tag="idxj",
                                               name=f"idx{e}_{j}", bufs=8)
                        nc.vector.tensor_copy(idxj[:], pslice[:, 0:1])
                        idx_i.append(idxj)
                        pgw.append(pslice)
                        xgj = xg_pool.tile([P, D], bf16, tag="xg", name=f"xg{e}_{j}")
                        nc.gpsimd.indirect_dma_start(
                            out=xgj[:],
                            out_offset=None,
                            in_=x[:, :],
                            in_offset=bass.IndirectOffsetOnAxis(
                                ap=idxj[:, :1], axis=0
                            ),
                            bounds_check=T - 1,
                            oob_is_err=False,
                        )
                        xg.append(xgj)

                    # prefetch next expert's weights (gpsimd queue, behind
                    # this expert's gathers but ahead of later experts')
                    if e + 1 < E:
                        emit_wloads(e + 1)

                    # PE-transpose gathered rows to d-major, batched copies.
                    # xgt layout [p=d%128, (dc, j, s)]
                    xgt = xgt_pool.tile([P, NDC * NCC * P], bf16, tag="xgt",
                                        name=f"xgt{e}")
                    xgt_v = xgt[:].rearrange("p (d j s) -> p d j s", d=NDC, j=NCC)
                    for j in range(NCC):
                        for dh in range(2):
                            ptr = ptr_pool.tile([P, TH], bf16)
                            for q in range(4):
                                dc = dh * 4 + q
                                nc.tensor.transpose(
                                    ptr[:, q * P : (q + 1) * P],
                                    xg[j][:, dc * P : (dc + 1) * P],
                                    ident_bf[:],
                                )
                            nc.vector.tensor_copy(
                                xgt_v[:, dh * 4 : (dh + 1) * 4, j, :],
                                ptr[:].rearrange("p (q s) -> p q s", q=4),
                            )

                    # phase A: hT = gelu(w1.T @ xgT + b1)
                    ht = [
                        h_pool.tile([P, CAP], bf16, tag=f"ht{fc}", name=f"ht{e}_{fc}")
                        for fc in range(NFC)
                    ]
                    for fc in range(NFC):
                        fsl = slice(fc * P, (fc + 1) * P)
                        ph = ph_pool.tile([P, CAP], f32)
                        for dc in range(NDC):
                            nc.tensor.matmul(
                                ph[:],
                                lhsT=w1_sb[dc][:, fsl],
                                rhs=xgt_v[:, dc, :, :],
                                start=(dc == 0),
                                stop=(dc == NDC - 1),
                            )
                        nc.scalar.activation(
                            ht[fc][:], ph[:], ACTF.Gelu, bias=b1_sb[:, fc : fc + 1]
                        )

                    # phase B: y = (hT.T @ w2) * p, scatter-add into out
                    for j in range(NCC):
                        jsl = slice(j * P, (j + 1) * P)
                        yst = yst_pool.tile([P, F], f32, tag="yst", name=f"yst{e}_{j}")
                        for gh in range(2):
                            gsl = slice(gh * TH, (gh + 1) * TH)
                            py = py_pool.tile([P, TH], f32)
                            for fc in range(NFC):
                                nc.tensor.matmul(
                                    py[:],
                                    lhsT=ht[fc][:, jsl],
                                    rhs=w2_sb[fc][:, gsl],
                                    start=(fc == 0),
                                    stop=(fc == NFC - 1),
                                )
                            if gh == 0:
                                nc.scalar.mul(yst[:, gsl], py[:], pgw[j][:, 1:2])
                            else:
                                nc.vector.tensor_scalar_mul(
                                    yst[:, gsl], py[:], pgw[j][:, 1:2]
                                )
                        nc.gpsimd.indirect_dma_start(
                            out=out[:, :],
                            out_offset=bass.IndirectOffsetOnAxis(
                                ap=idx_i[j][:, :1], axis=0
                            ),
                            in_=yst[:],
                            in_offset=None,
                            bounds_check=T - 1,
                            oob_is_err=False,
                            compute_op=ALU.add,
                        )

    nc.compile()
    return nc


_NC = None


def _get_nc():
    global _NC
    if _NC is None:
        _NC = build_moe()
    return _NC


def _maybe_enable_trace():
    """Register the NTFF profile hook (missing antenv.axon_hooks shim)."""
    try:
        import antenv

        if "antenv.axon_hooks" not in sys.modules:
            hooks = types.ModuleType("antenv.axon_hooks")
            hooks._hook = None
            hooks.set_axon_ntff_profile_hook = lambda h: setattr(hooks, "_hook", h)
            hooks.get_axon_ntff_profile_hook = lambda: hooks._hook
            sys.modules["antenv.axon_hooks"] = hooks
            antenv.axon_hooks = hooks
            from trn_agent_boot.trn_boot import _ntff_profile_via_ctypes

            hooks.set_axon_ntff_profile_hook(
                _ntff_profile_via_ctypes("/opt/axon/libaxon_pjrt.so")
            )
        return True
    except Exception:
        return False


LAST_EXEC_TIME_NS = None
LAST_RESULT = None


def kernel(x, gate_w, w1, b1, w2, b2):
    global LAST_EXEC_TIME_NS, LAST_RESULT
    x = np.ascontiguousarray(np.asarray(x, dtype=np.float32))
    gate_w = np.ascontiguousarray(np.asarray(gate_w, dtype=np.float32))
    w1 = np.ascontiguousarray(np.asarray(w1, dtype=np.float32))
    b1 = np.ascontiguousarray(np.asarray(b1, dtype=np.float32))
    w2 = np.ascontiguousarray(np.asarray(w2, dtype=np.float32))
    b2 = np.ascontiguousarray(np.asarray(b2, dtype=np.float32))

    trace = bool(os.environ.get("BASS_MOE_TRACE"))
    if trace:
        trace = _maybe_enable_trace()

    nc = _get_nc()
    in_maps = []
    for c in range(8):
        xs = x[c * T : (c + 1) * T]
        in_maps.append(
            {
                "x": xs,
                "xt": np.ascontiguousarray(xs.T),
                "gatew": gate_w,
                "w1": w1,
                "b1": b1,
                "w2": w2,
                "b2": b2,
            }
        )
    res = run_bass_kernel_spmd(nc, in_maps, list(range(8)), trace=trace)
    LAST_RESULT = res
    LAST_EXEC_TIME_NS = res.exec_time_ns
    out = np.concatenate([res.results[c]["out"] for c in range(8)], axis=0)
    gws = np.concatenate([res.results[c]["gw"] for c in range(8)], axis=0)
    return out, gws
